# revision 26
# baseline (speedup 1.0000x reference)
"""DiscreteBKI update kernel for Trainium2 (8 NeuronCores, Bass/Tile).

Pipeline (per core, x-slab of 32 planes + 1-plane halo each side):
  1. host: bucket valid points by x-plane, aggregate duplicate cells, and
     emit per-(plane, rz-row) scatter lists (col, count).
  2. device: histogram planes via gpsimd.local_scatter (per-partition
     indexed scatter of final counts into the blocked plane layout).
  3. device: 3x3x3 conv as banded matmuls per output plane over a
     (y%4, z) x (y//4, class) blocked layout; the y%4-block boundary terms
     use fx-stacked rotating edge tiles (2 matmuls/chunk instead of 6).
  4. device: fused psum + current_map add -> fp16 out planes; host casts.

Layout: y = 4g + r;  SBUF partition p = r*32 + z;  free col f = g*21 + c.
"""

import os
import sys

import numpy as np

for _p in (
    "/opt/trn_rl_repo",
    "/root/.axon_site/_ro/trn_rl_repo",
    "/root/.axon_site",
    "/root/.axon_site/_ro/pypackages",
):
    if os.path.isdir(_p) and _p not in sys.path:
        sys.path.append(_p)

import concourse.bacc as bacc  # noqa: E402
import concourse.mybir as mybir  # noqa: E402
import concourse.tile as tile  # noqa: E402
from concourse.bass_utils import run_bass_kernel_spmd  # noqa: E402

F16 = mybir.dt.float16
F32 = mybir.dt.float32
I16 = mybir.dt.int16
AF = mybir.ActivationFunctionType
ALU = mybir.AluOpType

# ---- problem geometry (hardcoded; must match the reference) ----
GX, GY, GZ, NC = 256, 256, 32, 21
MIN_B = np.array([-25.6, -25.6, -2.0], np.float32)
MAX_B = np.array([25.6, 25.6, 1.2], np.float32)
VOX = (MAX_B - MIN_B) / np.array([GX, GY, GZ], np.float32)
N_CORES = 8
XS = GX // N_CORES            # 32 x-planes owned per core
XL = XS + 2                   # 34 hist planes (with +-1 halo)
NI = 40                       # scatter slots per (plane, rz-row)
FREE = (GY // 4) * NC         # 1344
PAD = NC                      # 21 zero cols each side of a plane tile
PLANE_F = FREE + 2 * PAD      # 1386
RING_N = 12                   # ring buffer depth (hist planes in flight)
CHUNKS = ((0, 512), (512, 512), (1024, FREE - 1024))


def _build_masks():
    """Constant selection masks for assembling banded conv stationaries."""
    p = np.arange(128)
    r_in, z_in = p >> 5, p & 31
    r_out, z_out = r_in, z_in
    mask9 = np.zeros((128, 9, 128), np.float16)
    for fy in range(3):
        for fz in range(3):
            mask9[:, fy * 3 + fz, :] = (
                (r_in[:, None] - r_out[None, :] == fy - 1)
                & (z_in[:, None] - z_out[None, :] == fz - 1)
            )
    # edge mask: rows (b, z') with b = fx-block of the stacked edge tile
    pe = np.arange(96)
    be, ze = pe // 32, pe % 32
    zo = np.arange(32)
    maskE = np.zeros((96, 9, 32), np.float16)
    for b in range(3):
        for fz in range(3):
            maskE[:, b * 3 + fz, :] = (
                (be[:, None] == b) & (ze[:, None] - zo[None, :] == fz - 1)
            )
    return mask9.reshape(128, 9 * 128), maskE.reshape(96, 3 * 96)


def build_nc(reps: int = 1, ablate: frozenset = frozenset()):
    # ablate options (timing experiments only; results become wrong):
    #   'cross' - skip cross conv matmuls; 'main' - 1 main matmul only;
    #   'scat' - skip local_scatter; 'add' - skip final adds
    nc = bacc.Bacc(None, target_bir_lowering=False)

    map_t = nc.dram_tensor("map_blk", [XS, 128, FREE], F16, kind="ExternalInput")
    sidx_t = nc.dram_tensor("sidx", [128, XL * NI], I16, kind="ExternalInput")
    sval_t = nc.dram_tensor("sval", [128, XL * NI], F16, kind="ExternalInput")
    m0_t = nc.dram_tensor("m0w", [128, 3 * 128], F16, kind="ExternalInput")
    mpm_t = nc.dram_tensor("mpmw", [96, 64], F16, kind="ExternalInput")
    out_t = nc.dram_tensor("out_blk", [XS, 128, FREE], F16, kind="ExternalOutput")

    with tile.TileContext(nc) as tc:
        with (
            tc.tile_pool(name="const", bufs=1) as cp,
            tc.tile_pool(name="mapio", bufs=10) as mapp,
            tc.tile_pool(name="edge", bufs=16) as edgep,
            tc.tile_pool(name="cpm", bufs=8, space="PSUM") as cpp,
        ):
            # ---- constants ----
            sidx_sb = cp.tile([128, XL * NI], I16)
            sval_sb = cp.tile([128, XL * NI], F16)
            nc.sync.dma_start(out=sidx_sb[:], in_=sidx_t[:])
            nc.sync.dma_start(out=sval_sb[:], in_=sval_t[:])
            # host-prebuilt banded stationaries: m0[fx] 128x128, MP/MM 96x32
            m0_all = cp.tile([128, 3 * 128], F16)
            nc.sync.dma_start(out=m0_all[:], in_=m0_t[:])
            m0 = [m0_all[:, fx * 128: (fx + 1) * 128] for fx in range(3)]
            mpm = cp.tile([96, 64], F16)
            nc.sync.dma_start(out=mpm[:], in_=mpm_t[:])
            MP = mpm[:, 0:32]
            MM = mpm[:, 32:64]

            # persistent ring of hist planes
            ring_bufs = [cp.tile([128, PLANE_F], F16, name=f"ring{i}")
                         for i in range(RING_N)]

            def one_pass():
                ep_tiles: dict[int, object] = {}
                em_tiles: dict[int, object] = {}
                map_tiles: dict[int, object] = {}
                L = 6   # scatter runs L+2 planes ahead of the conv
                for it in range(XL + L):
                    p = it            # plane to scatter this iteration
                    q = it - 2 - L    # out-plane to convolve this iteration
                    if q + 2 < XS and q + 2 >= 0:
                        map_sb2 = mapp.tile([128, FREE], F16,
                                            name=f"map_{q + 2}", tag="map")
                        nc.sync.dma_start(out=map_sb2[:], in_=map_t[q + 2])
                        map_tiles[q + 2] = map_sb2
                    if p < XL:

                        # ---- histogram plane p via local scatter ----
                        ring_t = ring_bufs[p % RING_N]
                        if 'scat' in ablate:
                            nc.gpsimd.memset(ring_t[:], 0)
                        else:
                            # dst must be offset-0 (offset APs mis-scatter
                            # on hw); host bakes +PAD into the col indices,
                            # and the scatter's dst zeroing refreshes pads
                            nc.gpsimd.local_scatter(
                                out_ap=ring_t[:],
                                data_ap=sval_sb[:, p * NI: (p + 1) * NI],
                                idxs_ap=sidx_sb[:, p * NI: (p + 1) * NI],
                                channels=128, num_elems=PLANE_F, num_idxs=NI,
                            )
                        # stack edge rows into per-out-plane fx-stacked tiles:
                        # EP_q block b (partitions 32b:32b+32) holds plane q+b
                        if p < XS:
                            ep_tiles[p] = edgep.tile(
                                [96, PLANE_F], F16, name=f"EP_{p}", tag="EP")
                            em_tiles[p] = edgep.tile(
                                [96, PLANE_F], F16, name=f"EM_{p}", tag="EM")
                        for b in range(3):
                            tq = p - b
                            if 0 <= tq < XS:
                                nc.sync.dma_start(
                                    out=ep_tiles[tq][32 * b: 32 * b + 32, :],
                                    in_=ring_t[0:32, :])
                                nc.scalar.dma_start(
                                    out=em_tiles[tq][32 * b: 32 * b + 32, :],
                                    in_=ring_t[96:128, :])

                    # ---- conv + map add for out-plane q ----
                    if q < 0:
                        continue
                    map_sb = map_tiles.pop(q)
                    EP = ep_tiles.pop(q)
                    EM = em_tiles.pop(q)
                    cps = [cpp.tile([128, 512], F32, name=f"cp_{q}_{j}", tag="cp")
                           for j in range(3)]
                    nmain = 3 if 'main' not in ablate else 1
                    do_cross = 'cross' not in ablate
                    # chunk-outer: each chunk's psum group closes early so
                    # its drain overlaps the next chunk's matmuls
                    for j, (off, w) in enumerate(CHUNKS):
                        for fx in range(nmain):
                            nc.tensor.matmul(
                                out=cps[j][:, 0:w],
                                lhsT=m0[fx],
                                rhs=ring_bufs[(q + fx) % RING_N][
                                    :, PAD + off: PAD + off + w],
                                start=(fx == 0),
                                stop=(not do_cross and fx == nmain - 1),
                                skip_group_check=True,
                            )
                        if do_cross:
                            nc.tensor.matmul(
                                out=cps[j][96:128, 0:w],
                                lhsT=MP,
                                rhs=EP[0:96, PAD + off + 21: PAD + off + 21 + w],
                                start=False, stop=False,
                                tile_position=(0, 96),
                                skip_group_check=True,
                            )
                            nc.tensor.matmul(
                                out=cps[j][0:32, 0:w],
                                lhsT=MM,
                                rhs=EM[0:96, PAD + off - 21: PAD + off - 21 + w],
                                start=False, stop=True,
                                tile_position=(0, 0),
                                skip_group_check=True,
                            )
                    out_sb = mapp.tile([128, FREE], F16, tag="osb")
                    if 'add' not in ablate:
                        for j, (off, w) in enumerate(CHUNKS):
                            nc.vector.tensor_tensor(
                                out=out_sb[:, off: off + w],
                                in0=cps[j][:, 0:w],
                                in1=map_sb[:, off: off + w],
                                op=ALU.add,
                            )
                    nc.scalar.dma_start(out=out_t[q], in_=out_sb[:])

            for _rep in range(reps):
                one_pass()
    nc.compile()
    return nc


# ---------------- host side ----------------

_NC_CACHE: dict[int, object] = {}
LAST_EXEC_NS = None


def _get_nc(reps: int = 1):
    if reps not in _NC_CACHE:
        _NC_CACHE[reps] = build_nc(reps)
    return _NC_CACHE[reps]


def _build_stationaries(weights):
    """Banded conv stationaries from the 27 sigmoid weights (host-side)."""
    filt = 1.0 / (1.0 + np.exp(-weights.reshape(3, 3, 3).astype(np.float64)))
    filt = filt.astype(np.float32)
    filt[1, 1, 1] = 1.0
    mask9, maskE = _build_masks()
    m9 = mask9.reshape(128, 9, 128).astype(np.float32)
    mE = maskE.reshape(96, 9, 32).astype(np.float32)
    m0 = np.zeros((3, 128, 128), np.float32)
    for fx in range(3):
        for fy in range(3):
            for fz in range(3):
                m0[fx] += filt[fx, fy, fz] * m9[:, fy * 3 + fz]
    mpm = np.zeros((96, 64), np.float32)
    for b in range(3):
        for fz in range(3):
            mpm[:, 0:32] += filt[b, 2, fz] * mE[:, b * 3 + fz]
            mpm[:, 32:64] += filt[b, 0, fz] * mE[:, b * 3 + fz]
    m0q = np.ascontiguousarray(m0.transpose(1, 0, 2)).reshape(128, 3 * 128)
    return m0q.astype(np.float16), mpm.astype(np.float16)


def _prep_inputs(current_map, point_cloud, weights):
    """Compute per-core in_maps + overflow list on the host."""
    m0w, mpmw = _build_stationaries(weights)

    # blocked map: [x, (r,z), (g,c)]
    mb = np.ascontiguousarray(
        current_map.reshape(GX, GY // 4, 4, GZ, NC).transpose(0, 2, 3, 1, 4)
    ).reshape(GX, 128, FREE).astype(np.float16)

    xyz = point_cloud[:, :3]
    valid = np.all((xyz < MAX_B) & (xyz >= MIN_B), axis=1)
    inds = np.floor((xyz - MIN_B) / VOX).astype(np.int32)
    np.clip(inds, 0, np.array([GX - 1, GY - 1, GZ - 1], np.int32), out=inds)
    lab = np.clip(point_cloud[:, 3].astype(np.int32), 0, NC - 1)
    ix = inds[valid, 0]
    iy = inds[valid, 1]
    iz = inds[valid, 2]
    lab = lab[valid]

    a_all = (iy % 4) * 32 + iz                      # partition row
    col_all = (iy // 4) * NC + lab                  # free col

    in_maps = []
    overflow = []
    for c in range(N_CORES):
        x0 = XS * c
        sel = (ix >= x0 - 1) & (ix <= x0 + XS)
        plane = (ix[sel] - (x0 - 1)).astype(np.int64)
        key = (plane * 128 + a_all[sel]) * FREE + col_all[sel]
        uk, cnt = np.unique(key, return_counts=True)
        row = uk // FREE                             # plane*128 + a
        col_u = (uk % FREE).astype(np.int16)
        # rank within each row (uk sorted -> rows grouped)
        starts = np.flatnonzero(np.r_[True, row[1:] != row[:-1]])
        rank = np.arange(len(uk)) - np.repeat(
            starts, np.diff(np.r_[starts, len(uk)]))
        ok = rank < NI
        a_u = (row % 128).astype(np.int64)
        p_u = row // 128
        sidx = np.full((128, XL * NI), -1, np.int16)
        sval = np.zeros((128, XL * NI), np.float16)
        sidx[a_u[ok], p_u[ok] * NI + rank[ok]] = col_u[ok] + PAD
        sval[a_u[ok], p_u[ok] * NI + rank[ok]] = cnt[ok]
        if not ok.all():
            for k_, n_ in zip(uk[~ok], cnt[~ok]):
                r_, cl_ = divmod(int(k_), FREE)
                pl_, au_ = divmod(r_, 128)
                g_, lab_ = divmod(cl_, NC)
                oy = g_ * 4 + au_ // 32
                oz = au_ % 32
                ox = x0 - 1 + pl_
                overflow.append((c, ox, oy, oz, lab_, int(n_)))
        in_maps.append(
            {
                "map_blk": np.ascontiguousarray(mb[x0: x0 + XS]),
                "sidx": sidx,
                "sval": sval,
                "m0w": m0w,
                "mpmw": mpmw,
            }
        )
    return in_maps, overflow


def _apply_overflow(out, overflow, weights):
    if not overflow:
        return
    filt = 1.0 / (1.0 + np.exp(-weights.reshape(3, 3, 3).astype(np.float64)))
    filt = filt.astype(np.float32)
    filt[1, 1, 1] = 1.0
    for c, ix, iy, iz, lab, n in overflow:
        x0, x1 = XS * c, XS * (c + 1)
        for k0 in range(3):
            ox = ix + 1 - k0
            if ox < x0 or ox >= x1:
                continue
            for k1 in range(3):
                oy = iy + 1 - k1
                if oy < 0 or oy >= GY:
                    continue
                for k2 in range(3):
                    oz = iz + 1 - k2
                    if oz < 0 or oz >= GZ:
                        continue
                    out[ox, oy, oz, lab] += n * filt[k0, k1, k2]


def kernel(current_map, point_cloud, weights):
    global LAST_EXEC_NS
    current_map = np.asarray(current_map, np.float32)
    point_cloud = np.asarray(point_cloud, np.float32)
    weights = np.asarray(weights, np.float32)

    nc = _get_nc(1)
    in_maps, overflow = _prep_inputs(current_map, point_cloud, weights)
    res = run_bass_kernel_spmd(nc, in_maps, core_ids=list(range(N_CORES)))
    LAST_EXEC_NS = res.exec_time_ns

    out = np.empty((GX, GY, GZ, NC), np.float32)
    for c in range(N_CORES):
        blk = res.results[c]["out_blk"].astype(np.float32)  # [32, 128, 1344]
        out[XS * c: XS * (c + 1)] = (
            blk.reshape(XS, 4, 32, GY // 4, NC)
            .transpose(0, 3, 1, 2, 4)
            .reshape(XS, GY, GZ, NC)
        )
    _apply_overflow(out, overflow, weights)
    return out


# revision 28
# speedup vs baseline: 1.0110x; 1.0110x over previous
"""DiscreteBKI update kernel for Trainium2 (8 NeuronCores, Bass/Tile).

Pipeline (per core, x-slab of 32 planes + 1-plane halo each side):
  1. host: bucket valid points by x-plane, aggregate duplicate cells, and
     emit per-(plane, rz-row) scatter lists (col, count).
  2. device: histogram planes via gpsimd.local_scatter (per-partition
     indexed scatter of final counts into the blocked plane layout).
  3. device: 3x3x3 conv as banded matmuls per output plane over a
     (y%4, z) x (y//4, class) blocked layout; the y%4-block boundary terms
     use fx-stacked rotating edge tiles (2 matmuls/chunk instead of 6).
  4. device: fused psum + current_map add -> fp16 out planes; host casts.

Layout: y = 4g + r;  SBUF partition p = r*32 + z;  free col f = g*21 + c.
"""

import os
import sys

import numpy as np

for _p in (
    "/opt/trn_rl_repo",
    "/root/.axon_site/_ro/trn_rl_repo",
    "/root/.axon_site",
    "/root/.axon_site/_ro/pypackages",
):
    if os.path.isdir(_p) and _p not in sys.path:
        sys.path.append(_p)

import concourse.bacc as bacc  # noqa: E402
import concourse.mybir as mybir  # noqa: E402
import concourse.tile as tile  # noqa: E402
from concourse.bass_utils import run_bass_kernel_spmd  # noqa: E402

F16 = mybir.dt.float16
F32 = mybir.dt.float32
I16 = mybir.dt.int16
AF = mybir.ActivationFunctionType
ALU = mybir.AluOpType

# ---- problem geometry (hardcoded; must match the reference) ----
GX, GY, GZ, NC = 256, 256, 32, 21
MIN_B = np.array([-25.6, -25.6, -2.0], np.float32)
MAX_B = np.array([25.6, 25.6, 1.2], np.float32)
VOX = (MAX_B - MIN_B) / np.array([GX, GY, GZ], np.float32)
N_CORES = 8
XS = GX // N_CORES            # 32 x-planes owned per core
XL = XS + 2                   # 34 hist planes (with +-1 halo)
NI = 40                       # scatter slots per (plane, rz-row)
FREE = (GY // 4) * NC         # 1344
PAD = NC                      # 21 zero cols each side of a plane tile
PLANE_F = FREE + 2 * PAD      # 1386
RING_N = 6                    # ring buffer depth per half-stream
CHUNKS = ((0, 512), (512, 512), (1024, FREE - 1024))


def _build_masks():
    """Constant selection masks for assembling banded conv stationaries."""
    p = np.arange(128)
    r_in, z_in = p >> 5, p & 31
    r_out, z_out = r_in, z_in
    mask9 = np.zeros((128, 9, 128), np.float16)
    for fy in range(3):
        for fz in range(3):
            mask9[:, fy * 3 + fz, :] = (
                (r_in[:, None] - r_out[None, :] == fy - 1)
                & (z_in[:, None] - z_out[None, :] == fz - 1)
            )
    # edge mask: rows (b, z') with b = fx-block of the stacked edge tile
    pe = np.arange(96)
    be, ze = pe // 32, pe % 32
    zo = np.arange(32)
    maskE = np.zeros((96, 9, 32), np.float16)
    for b in range(3):
        for fz in range(3):
            maskE[:, b * 3 + fz, :] = (
                (be[:, None] == b) & (ze[:, None] - zo[None, :] == fz - 1)
            )
    return mask9.reshape(128, 9 * 128), maskE.reshape(96, 3 * 96)


def build_nc(reps: int = 1, ablate: frozenset = frozenset()):
    # ablate options (timing experiments only; results become wrong):
    #   'cross' - skip cross conv matmuls; 'main' - 1 main matmul only;
    #   'scat' - skip local_scatter; 'add' - skip final adds
    nc = bacc.Bacc(None, target_bir_lowering=False)

    map_t = nc.dram_tensor("map_blk", [XS, 128, FREE], F16, kind="ExternalInput")
    sidx_t = nc.dram_tensor("sidx", [128, XL * NI], I16, kind="ExternalInput")
    sval_t = nc.dram_tensor("sval", [128, XL * NI], F16, kind="ExternalInput")
    m0_t = nc.dram_tensor("m0w", [128, 3 * 128], F16, kind="ExternalInput")
    mpm_t = nc.dram_tensor("mpmw", [96, 64], F16, kind="ExternalInput")
    out_t = nc.dram_tensor("out_blk", [XS, 128, FREE], F16, kind="ExternalOutput")

    with tile.TileContext(nc) as tc:
        with (
            tc.tile_pool(name="const", bufs=1) as cp,
            tc.tile_pool(name="mapio", bufs=8) as mapp,
            tc.tile_pool(name="edge", bufs=16) as edgep,
            tc.tile_pool(name="cpm", bufs=8, space="PSUM") as cpp,
        ):
            # ---- constants ----
            sidx_sb = cp.tile([128, XL * NI], I16)
            sval_sb = cp.tile([128, XL * NI], F16)
            nc.sync.dma_start(out=sidx_sb[:], in_=sidx_t[:])
            nc.sync.dma_start(out=sval_sb[:], in_=sval_t[:])
            # host-prebuilt banded stationaries: m0[fx] 128x128, MP/MM 96x32
            m0_all = cp.tile([128, 3 * 128], F16)
            nc.sync.dma_start(out=m0_all[:], in_=m0_t[:])
            m0 = [m0_all[:, fx * 128: (fx + 1) * 128] for fx in range(3)]
            mpm = cp.tile([96, 64], F16)
            nc.sync.dma_start(out=mpm[:], in_=mpm_t[:])
            MP = mpm[:, 0:32]
            MM = mpm[:, 32:64]

            # two independent half-slab streams: ring buffers per half
            ring_bufs = [[cp.tile([128, PLANE_F], F16, name=f"ring{h}_{i}")
                          for i in range(RING_N)] for h in range(2)]

            XH = XS // 2          # 16 out-planes per half
            XLH = XH + 2          # 18 hist planes per half

            def one_pass():
                ep_tiles: dict[tuple, object] = {}
                em_tiles: dict[tuple, object] = {}
                map_tiles: dict[int, object] = {}
                L = 2   # scatter runs L+2 planes ahead of the conv per half
                for i in range(XLH + 2 + L):
                    for h in range(2):
                        base = XH * h
                        p = i            # local hist plane
                        q = i - 2 - L    # local out plane
                        m = i - L        # local map plane to prefetch
                        if 0 <= m < XH:
                            gm = base + m
                            map_sb2 = mapp.tile([128, FREE], F16,
                                                name=f"map_{gm}", tag="map")
                            nc.sync.dma_start(out=map_sb2[:], in_=map_t[gm])
                            map_tiles[gm] = map_sb2
                        if p < XLH:
                            gp = base + p     # global hist plane (0..33)
                            ring_t = ring_bufs[h][p % RING_N]
                            if 'scat' in ablate:
                                nc.gpsimd.memset(ring_t[:], 0)
                            else:
                                # dst must be offset-0 (offset APs mis-scatter
                                # on hw); host bakes +PAD into col indices,
                                # and the dst zeroing refreshes the pads
                                nc.gpsimd.local_scatter(
                                    out_ap=ring_t[:],
                                    data_ap=sval_sb[:, gp * NI: (gp + 1) * NI],
                                    idxs_ap=sidx_sb[:, gp * NI: (gp + 1) * NI],
                                    channels=128, num_elems=PLANE_F,
                                    num_idxs=NI,
                                )
                            # stack edge rows into per-out-plane tiles:
                            # EP_q block b (partitions 32b:+32) = plane q+b
                            if p < XH:
                                ep_tiles[h, p] = edgep.tile(
                                    [96, PLANE_F], F16,
                                    name=f"EP_{h}_{p}", tag="EP")
                                em_tiles[h, p] = edgep.tile(
                                    [96, PLANE_F], F16,
                                    name=f"EM_{h}_{p}", tag="EM")
                            for b in range(3):
                                tq = p - b
                                if 0 <= tq < XH:
                                    nc.sync.dma_start(
                                        out=ep_tiles[h, tq][
                                            32 * b: 32 * b + 32, :],
                                        in_=ring_t[0:32, :])
                                    nc.scalar.dma_start(
                                        out=em_tiles[h, tq][
                                            32 * b: 32 * b + 32, :],
                                        in_=ring_t[96:128, :])

                        # ---- conv + map add for out-plane base+q ----
                        if q < 0 or q >= XH:
                            continue
                        gq = base + q
                        map_sb = map_tiles.pop(gq)
                        EP = ep_tiles.pop((h, q))
                        EM = em_tiles.pop((h, q))
                        cps = [cpp.tile([128, 512], F32,
                                        name=f"cp_{gq}_{j}", tag="cp")
                               for j in range(3)]
                        nmain = 3 if 'main' not in ablate else 1
                        do_cross = 'cross' not in ablate
                        for j, (off, w) in enumerate(CHUNKS):
                            for fx in range(nmain):
                                nc.tensor.matmul(
                                    out=cps[j][:, 0:w],
                                    lhsT=m0[fx],
                                    rhs=ring_bufs[h][(q + fx) % RING_N][
                                        :, PAD + off: PAD + off + w],
                                    start=(fx == 0),
                                    stop=(not do_cross and fx == nmain - 1),
                                    skip_group_check=True,
                                )
                            if do_cross:
                                nc.tensor.matmul(
                                    out=cps[j][96:128, 0:w],
                                    lhsT=MP,
                                    rhs=EP[0:96,
                                           PAD + off + 21: PAD + off + 21 + w],
                                    start=False, stop=False,
                                    tile_position=(0, 96),
                                    skip_group_check=True,
                                )
                                nc.tensor.matmul(
                                    out=cps[j][0:32, 0:w],
                                    lhsT=MM,
                                    rhs=EM[0:96,
                                           PAD + off - 21: PAD + off - 21 + w],
                                    start=False, stop=True,
                                    tile_position=(0, 0),
                                    skip_group_check=True,
                                )
                        out_sb = mapp.tile([128, FREE], F16,
                                           name=f"osb_{gq}", tag="osb")
                        if 'add' not in ablate:
                            for j, (off, w) in enumerate(CHUNKS):
                                nc.vector.tensor_tensor(
                                    out=out_sb[:, off: off + w],
                                    in0=cps[j][:, 0:w],
                                    in1=map_sb[:, off: off + w],
                                    op=ALU.add,
                                )
                        nc.scalar.dma_start(out=out_t[gq], in_=out_sb[:])

            for _rep in range(reps):
                one_pass()
    nc.compile()
    return nc


# ---------------- host side ----------------

_NC_CACHE: dict[int, object] = {}
LAST_EXEC_NS = None


def _get_nc(reps: int = 1):
    if reps not in _NC_CACHE:
        _NC_CACHE[reps] = build_nc(reps)
    return _NC_CACHE[reps]


def _build_stationaries(weights):
    """Banded conv stationaries from the 27 sigmoid weights (host-side)."""
    filt = 1.0 / (1.0 + np.exp(-weights.reshape(3, 3, 3).astype(np.float64)))
    filt = filt.astype(np.float32)
    filt[1, 1, 1] = 1.0
    mask9, maskE = _build_masks()
    m9 = mask9.reshape(128, 9, 128).astype(np.float32)
    mE = maskE.reshape(96, 9, 32).astype(np.float32)
    m0 = np.zeros((3, 128, 128), np.float32)
    for fx in range(3):
        for fy in range(3):
            for fz in range(3):
                m0[fx] += filt[fx, fy, fz] * m9[:, fy * 3 + fz]
    mpm = np.zeros((96, 64), np.float32)
    for b in range(3):
        for fz in range(3):
            mpm[:, 0:32] += filt[b, 2, fz] * mE[:, b * 3 + fz]
            mpm[:, 32:64] += filt[b, 0, fz] * mE[:, b * 3 + fz]
    m0q = np.ascontiguousarray(m0.transpose(1, 0, 2)).reshape(128, 3 * 128)
    return m0q.astype(np.float16), mpm.astype(np.float16)


def _prep_inputs(current_map, point_cloud, weights):
    """Compute per-core in_maps + overflow list on the host."""
    m0w, mpmw = _build_stationaries(weights)

    # blocked map: [x, (r,z), (g,c)]
    mb = np.ascontiguousarray(
        current_map.reshape(GX, GY // 4, 4, GZ, NC).transpose(0, 2, 3, 1, 4)
    ).reshape(GX, 128, FREE).astype(np.float16)

    xyz = point_cloud[:, :3]
    valid = np.all((xyz < MAX_B) & (xyz >= MIN_B), axis=1)
    inds = np.floor((xyz - MIN_B) / VOX).astype(np.int32)
    np.clip(inds, 0, np.array([GX - 1, GY - 1, GZ - 1], np.int32), out=inds)
    lab = np.clip(point_cloud[:, 3].astype(np.int32), 0, NC - 1)
    ix = inds[valid, 0]
    iy = inds[valid, 1]
    iz = inds[valid, 2]
    lab = lab[valid]

    a_all = (iy % 4) * 32 + iz                      # partition row
    col_all = (iy // 4) * NC + lab                  # free col

    in_maps = []
    overflow = []
    for c in range(N_CORES):
        x0 = XS * c
        sel = (ix >= x0 - 1) & (ix <= x0 + XS)
        plane = (ix[sel] - (x0 - 1)).astype(np.int64)
        key = (plane * 128 + a_all[sel]) * FREE + col_all[sel]
        uk, cnt = np.unique(key, return_counts=True)
        row = uk // FREE                             # plane*128 + a
        col_u = (uk % FREE).astype(np.int16)
        # rank within each row (uk sorted -> rows grouped)
        starts = np.flatnonzero(np.r_[True, row[1:] != row[:-1]])
        rank = np.arange(len(uk)) - np.repeat(
            starts, np.diff(np.r_[starts, len(uk)]))
        ok = rank < NI
        a_u = (row % 128).astype(np.int64)
        p_u = row // 128
        sidx = np.full((128, XL * NI), -1, np.int16)
        sval = np.zeros((128, XL * NI), np.float16)
        sidx[a_u[ok], p_u[ok] * NI + rank[ok]] = col_u[ok] + PAD
        sval[a_u[ok], p_u[ok] * NI + rank[ok]] = cnt[ok]
        if not ok.all():
            for k_, n_ in zip(uk[~ok], cnt[~ok]):
                r_, cl_ = divmod(int(k_), FREE)
                pl_, au_ = divmod(r_, 128)
                g_, lab_ = divmod(cl_, NC)
                oy = g_ * 4 + au_ // 32
                oz = au_ % 32
                ox = x0 - 1 + pl_
                overflow.append((c, ox, oy, oz, lab_, int(n_)))
        in_maps.append(
            {
                "map_blk": np.ascontiguousarray(mb[x0: x0 + XS]),
                "sidx": sidx,
                "sval": sval,
                "m0w": m0w,
                "mpmw": mpmw,
            }
        )
    return in_maps, overflow


def _apply_overflow(out, overflow, weights):
    if not overflow:
        return
    filt = 1.0 / (1.0 + np.exp(-weights.reshape(3, 3, 3).astype(np.float64)))
    filt = filt.astype(np.float32)
    filt[1, 1, 1] = 1.0
    for c, ix, iy, iz, lab, n in overflow:
        x0, x1 = XS * c, XS * (c + 1)
        for k0 in range(3):
            ox = ix + 1 - k0
            if ox < x0 or ox >= x1:
                continue
            for k1 in range(3):
                oy = iy + 1 - k1
                if oy < 0 or oy >= GY:
                    continue
                for k2 in range(3):
                    oz = iz + 1 - k2
                    if oz < 0 or oz >= GZ:
                        continue
                    out[ox, oy, oz, lab] += n * filt[k0, k1, k2]


def kernel(current_map, point_cloud, weights):
    global LAST_EXEC_NS
    current_map = np.asarray(current_map, np.float32)
    point_cloud = np.asarray(point_cloud, np.float32)
    weights = np.asarray(weights, np.float32)

    nc = _get_nc(1)
    in_maps, overflow = _prep_inputs(current_map, point_cloud, weights)
    res = run_bass_kernel_spmd(nc, in_maps, core_ids=list(range(N_CORES)))
    LAST_EXEC_NS = res.exec_time_ns

    out = np.empty((GX, GY, GZ, NC), np.float32)
    for c in range(N_CORES):
        blk = res.results[c]["out_blk"].astype(np.float32)  # [32, 128, 1344]
        out[XS * c: XS * (c + 1)] = (
            blk.reshape(XS, 4, 32, GY // 4, NC)
            .transpose(0, 3, 1, 2, 4)
            .reshape(XS, GY, GZ, NC)
        )
    _apply_overflow(out, overflow, weights)
    return out
